# revision 4
# baseline (speedup 1.0000x reference)
"""Self-attention kernel for Trainium2 (8 NeuronCores, data-parallel over batch).

Problem: x [8, 2048, 512] f32, mask [8, 2048] i32.
  scores = x @ x^T per batch; rows with mask==0 are fully masked (-1e9),
  softmax over last dim, out = alpha @ x.

Per-core algorithm (batch b on core b), S=2048, D=512:
  - Softmax shift: softmax(s - c) is shift-invariant per row, so instead of
    the row max we shift by d_m = ||x_m||^2 (the Gram diagonal), which is a
    numerically safe shift for the softmax exp (s_mj - d_m <= ~0.1 + eps).
  - Scores are computed TRANSPOSED (S^T[j, m], key j on partitions) so the
    softmax tiles feed the PV matmul directly as stationary operands with V
    in natural layout; the -d_m shift is folded into the matmul as a K=1
    accumulation row (ones lhsT x (-d) rhs outer product).
  - l_m (softmax denominator) comes from an N=2 ones-matmul sharing the PV
    stationary tiles; out rows are scaled by mask/l and masked rows are
    blended with the (uniform-softmax) mean row afterwards.
  - All matmuls run in float32r (full PE rate at N>=512, ~1.2e-4 relative
    accuracy).
"""

import numpy as np

import concourse.bacc as bacc
import concourse.mybir as mybir
from concourse.tile import TileContext
from concourse.bass_utils import run_bass_kernel_spmd
from concourse.masks import make_identity

F32 = mybir.dt.float32
F32R = mybir.dt.float32r
I32 = mybir.dt.int32
AF = mybir.ActivationFunctionType

B, S, D = 8, 2048, 512
P = 128
NT = S // P          # 16 sequence tiles
NC = D // P          # 4 contraction chunks
NMM = 4              # m-macros of 512 queries
MMW = S // NMM       # 512 queries per macro

_BUILT = None


def _build():
    nc = bacc.Bacc()
    x_ext = nc.dram_tensor("x", [S, D], F32, kind="ExternalInput")
    mask_ext = nc.dram_tensor("mask", [S], I32, kind="ExternalInput")
    out_ext = nc.dram_tensor("out", [S, D], F32, kind="ExternalOutput")

    with TileContext(nc) as tc:
        with (
            tc.tile_pool(name="const", bufs=1) as constp,
            tc.tile_pool(name="xr", bufs=1) as xrp,
            tc.tile_pool(name="xtr", bufs=1) as xtrp,
            tc.tile_pool(name="xin", bufs=3) as xinp,
            tc.tile_pool(name="pt", bufs=2) as ptp,
            tc.tile_pool(name="work", bufs=2) as wp,
            tc.tile_pool(name="outp", bufs=3) as outp,
        ):
            # ---- constants ----
            ident = constp.tile([P, P], F32R, name="ident")
            identf = constp.tile([P, P], F32, name="identf")
            make_identity(nc, identf[:])
            nc.vector.tensor_copy(ident[:], identf[:])

            ones_f = constp.tile([P, 2], F32, name="ones_f")
            nc.gpsimd.memset(ones_f[:], 1.0)
            ones2 = constp.tile([P, 2], F32R, name="ones2")      # PV l rhs
            nc.vector.tensor_copy(ones2[:], ones_f[:])
            ones1 = ones2[:, 0:1]                                 # colsum lhsT

            ones_rf = constp.tile([1, P], F32, name="ones_rf")
            nc.gpsimd.memset(ones_rf[:], 1.0)
            ones_row = constp.tile([1, P], F32R, name="ones_row")  # K=1 lhsT
            nc.vector.tensor_copy(ones_row[:], ones_rf[:])

            # Preload ACT tables (exp/square) so the ~2.7us table load
            # overlaps the input DMAs instead of stalling the first S^T tile.
            dummy = constp.tile([P, 2], F32, name="dummy")
            nc.scalar.activation(dummy[:], ones_f[:], AF.Exp)
            nc.scalar.activation(dummy[:], ones_f[:], AF.Square)

            # ---- phase A/B psum pools ----
            ab_pools = (
                tc.tile_pool(name="ps_tr", bufs=2, space="PSUM"),
                tc.tile_pool(name="ps_aux", bufs=1, space="PSUM"),
            )
            ps_tr = ab_pools[0].__enter__()
            ps_aux = ab_pools[1].__enter__()

            # ---- phase A: load x, cast to f32r; masks ----
            xr = [xrp.tile([P, D], F32R, name=f"xr{t}") for t in range(NT)]
            for t in range(NT):
                xf = xinp.tile([P, D], F32, name="xf", tag="xf")
                nc.sync.dma_start(out=xf[:], in_=x_ext[t * P:(t + 1) * P, :])
                nc.vector.tensor_copy(xr[t][:], xf[:])

            mi = constp.tile([P, NT], I32, name="mi")
            nc.sync.dma_start(out=mi[:], in_=mask_ext.rearrange("(t p) -> p t", p=P))
            maskf = constp.tile([P, NT], F32, name="maskf")
            nc.vector.tensor_copy(maskf[:], mi[:])
            invmaskf = constp.tile([P, NT], F32, name="invmaskf")
            nc.scalar.activation(invmaskf[:], maskf[:], AF.Copy, bias=1.0, scale=-1.0)

            # mean row: mean[d] = (1/S) sum_j x[j, d]
            ps_m = ps_aux.tile([1, D], F32, name="ps_m", tag="ps_m")
            for t in range(NT):
                nc.tensor.matmul(ps_m[:], ones1, xr[t][:], start=(t == 0), stop=(t == NT - 1))
            meanrow = constp.tile([1, D], F32R, name="meanrow")
            nc.vector.tensor_scalar_mul(meanrow[:], ps_m[:], 1.0 / S)

            # ---- phase B: transpose x -> XTr [d, seq] ----
            xtr = [xtrp.tile([P, S], F32R, name=f"xtr{c}") for c in range(NC)]
            for t in range(NT):
                for c in range(NC):
                    pt_ps = ps_tr.tile([P, P], F32R, name="pt_ps", tag="pt_ps")
                    nc.tensor.transpose(pt_ps[:], xr[t][:, c * P:(c + 1) * P], ident[:])
                    nc.vector.tensor_copy(xtr[c][:, t * P:(t + 1) * P], pt_ps[:])

            # negd[0, m] = -||x_m||^2 via ones-matmul over squared XTr chunks
            negd = constp.tile([1, S], F32R, name="negd")
            ps_d = [ps_aux.tile([1, MMW], F32, name=f"ps_d{s}", tag=f"ps_d{s}") for s in range(NMM)]
            for c in range(NC):
                xtsq = wp.tile([P, S], F32R, name="xtsq", tag="xtsq")
                nc.scalar.activation(xtsq[:], xtr[c][:], AF.Square)
                for s in range(NMM):
                    nc.tensor.matmul(ps_d[s][:], ones1, xtsq[:, s * MMW:(s + 1) * MMW],
                                     start=(c == 0), stop=(c == NC - 1))
            for s in range(NMM):
                nc.vector.tensor_scalar_mul(negd[0:1, s * MMW:(s + 1) * MMW], ps_d[s][:], -1.0)

            # mean broadcast [P, D] for masked-row blending
            ps_mb = ps_aux.tile([P, D], F32, name="ps_mb", tag="ps_mb")
            nc.tensor.matmul(ps_mb[:], ones_row[:], meanrow[:], start=True, stop=True)
            meanbc = constp.tile([P, D], F32, name="meanbc")
            nc.vector.tensor_copy(meanbc[:], ps_mb[:])

            ab_pools[1].__exit__(None, None, None)
            ab_pools[0].__exit__(None, None, None)

            c_pools = (
                tc.tile_pool(name="ps_s", bufs=3, space="PSUM"),
                tc.tile_pool(name="ps_o", bufs=2, space="PSUM"),
                tc.tile_pool(name="ps_l", bufs=2, space="PSUM"),
            )
            ps_s = c_pools[0].__enter__()
            ps_o = c_pools[1].__enter__()
            ps_l = c_pools[2].__enter__()

            # ---- phase C: attention per m-macro ----
            for mm in range(NMM):
                msl = slice(mm * MMW, (mm + 1) * MMW)
                # S^T tiles + exp -> pT[jc] [P j, MMW m] f32r
                pts = []
                for jc in range(NT):
                    pss = ps_s.tile([P, MMW], F32, name="pss", tag="pss")
                    for c in range(NC):
                        nc.tensor.matmul(pss[:], xtr[c][:, jc * P:(jc + 1) * P],
                                         xtr[c][:, msl], start=(c == 0), stop=False)
                    nc.tensor.matmul(pss[:], ones_row[:], negd[0:1, msl],
                                     start=False, stop=True)
                    pt = ptp.tile([P, MMW], F32R, name=f"pt{jc}", tag=f"pt{jc}")
                    nc.scalar.activation(pt[:], pss[:], AF.Exp)
                    pts.append(pt)

                # PV + l, normalize, blend, store
                for mt in range(NMM):
                    t = mm * NMM + mt
                    pso = ps_o.tile([P, D], F32, name="pso", tag="pso")
                    psl = ps_l.tile([P, 2], F32, name="psl", tag="psl")
                    for jc in range(NT):
                        lhs = pts[jc][:, mt * P:(mt + 1) * P]
                        nc.tensor.matmul(pso[:], lhs, xr[jc][:],
                                         start=(jc == 0), stop=(jc == NT - 1))
                        nc.tensor.matmul(psl[:], lhs, ones2[:],
                                         start=(jc == 0), stop=(jc == NT - 1))
                    rc = wp.tile([P, 1], F32, name="rc", tag="rc")
                    nc.vector.reciprocal(rc[:], psl[:, 0:1])
                    rcm = wp.tile([P, 1], F32, name="rcm", tag="rcm")
                    nc.vector.tensor_mul(rcm[:], rc[:], maskf[:, t:t + 1])
                    om = outp.tile([P, D], F32, name="om", tag="om")
                    nc.vector.tensor_scalar_mul(om[:], pso[:], rcm[:])
                    mb = outp.tile([P, D], F32, name="mb", tag="mb")
                    nc.scalar.activation(mb[:], meanbc[:], AF.Copy, scale=invmaskf[:, t:t + 1])
                    outt = outp.tile([P, D], F32, name="outt", tag="outt")
                    nc.vector.tensor_add(outt[:], om[:], mb[:])
                    nc.sync.dma_start(out=out_ext[t * P:(t + 1) * P, :], in_=outt[:])

            for cp in reversed(c_pools):
                cp.__exit__(None, None, None)

    nc.finalize()
    return nc


def kernel(x, mask):
    global _BUILT
    if _BUILT is None:
        _BUILT = _build()
    nc = _BUILT
    x = np.ascontiguousarray(np.asarray(x), dtype=np.float32)
    mask = np.ascontiguousarray(np.asarray(mask), dtype=np.int32)
    ins = [{"x": x[c], "mask": mask[c]} for c in range(B)]
    res = run_bass_kernel_spmd(nc, ins, list(range(B)))
    return np.stack([res.results[c]["out"] for c in range(B)], axis=0)
